# revision 15
# baseline (speedup 1.0000x reference)
"""ContrastiveLoss Trainium2 kernel (v2 — DVE tensor_scalar fast path).

Math (matches the jax reference):
    an = l2norm(inputs_col); bn = l2norm(inputs_row)
    sim = an @ bn.T                                     [n, n]
    same = targets_col[:,None] == target_row[None,:]
    pos = same & (sim < 1-1e-5);  neg = ~same & (sim > 0.5)
    loss = sum(where(any(pos,1), sum(pos*(1-sim) + neg*sim, 1), 0)) / n

Distribution: rows of inputs_col sharded across 8 cores (1024 each);
inputs_row / target_row replicated. Each core emits one fp32 partial sum;
the host adds the 8 partials and divides by n.

v2 design notes (from the v1 profile: GPSIMD tensor_scalar 19.3us/block and
DVE scalar_tensor_tensor 7.3us/block dominated — 3.9ms of a 3.18ms kernel):
  * All per-element mask math now uses DVE tensor_scalar (the only DVE op
    with 4x_2p uops; scalar_tensor_tensor has none) plus one tensor_tensor.
  * Additive masking replaces multiplicative masking:
        w = sim + BIG*(label mismatch)
    Then, exactly:
        pos_i  = sum_j same*relu(c - sim)    = -sum_j min(w - c, 0)
        negA_i = sum_j ~same*relu(sim - m)   =  sum_j max(w - (m+BIG), 0)
        negB_i = sum_j ~same*[sim > m]*m     =  sum_j (w > m+BIG)*m
        neg_i  = sum_j ~same*[sim>m]*sim     = negA_i + negB_i
        row_loss = (pos_i > 0) * (pos_i + neg_i)
    (uses v in [-1,1], BIG=8 > 1 + c and m+BIG-1 > 1, so the shifted branch
    can never fire for the wrong mask value)
  * Labels are broadcast as fp16 (exact for 0..1023; bf16 is NOT) so the
    not_equal tensor_scalar runs in a 2-byte 4x-capable layout.
  * Elementwise runs on [128, 2048] superblocks to amortize fixed op costs.
  * One blocked dma_start_transpose per [128,1024] tile (out [128,8,128])
    instead of 8 square transposes; DMA dispatches alternate sync/scalar.
"""

import numpy as np
from contextlib import ExitStack

import concourse.bass as bass
import concourse.mybir as mybir
import concourse.tile as tile
from concourse import bacc
from concourse.bass import ds, ts

N = 8192            # rows of inputs_col / inputs_row
D = 1024            # feature dim
NCORES = 8
ROWS = N // NCORES  # inputs_col rows per core
P = 128             # SBUF partitions
NCH = ROWS // P     # i-chunks per core (8)
KT = D // P         # contraction tiles (8)
QJ = 2048           # B-column quarter width == elementwise superblock width
NQ = N // QJ        # quarters (4)

EPS_NORM = 1e-12
EPS_POS = 1e-5
MARGIN = 0.5
BIG = 8.0

F32 = mybir.dt.float32
F16 = mybir.dt.float16
BF16 = mybir.dt.bfloat16
AF = mybir.ActivationFunctionType
OP = mybir.AluOpType


def _normalize_tile(nc, pools, dma_eng, x_f32, x_bf, eps_tile):
    """x_bf = bf16(x_f32 / sqrt(sum(x_f32^2, axis=1) + EPS_NORM)).

    Square+accum on ACT, rsqrt on ACT, cast+scale on DVE tensor_scalar
    (2x_2p for fp32 single-src)."""
    small, junk = pools
    sq = small.tile([P, 1], F32, tag="sq")
    sqj = junk.tile([P, D], BF16, tag="sqj")
    nc.scalar.activation(sqj, x_f32, AF.Square, accum_out=sq)
    nc.scalar.activation(sq, sq, AF.Sqrt, bias=eps_tile)
    inv = small.tile([P, 1], F32, tag="inv")
    nc.vector.reciprocal(inv, sq)
    # cast+scale on DVE (2x_2p for fp32 single-src) to offload ACT
    nc.vector.tensor_scalar(
        out=x_bf, in0=x_f32, scalar1=inv, scalar2=None, op0=OP.mult
    )


def build_kernel_body(tc, out_ap, a_ap, b_ap, tcol_ap, trow_ap):
    nc = tc.nc
    dma_engines = [nc.sync, nc.scalar]

    ctx = ExitStack()
    with ctx:
        singles = ctx.enter_context(tc.tile_pool(name="singles", bufs=1))
        small = ctx.enter_context(tc.tile_pool(name="small", bufs=6))
        junk = ctx.enter_context(tc.tile_pool(name="junk", bufs=2))
        stage_f32 = ctx.enter_context(tc.tile_pool(name="stage_f32", bufs=3))
        stage_bf = ctx.enter_context(tc.tile_pool(name="stage_bf", bufs=4))
        # 3 resident B quarters: prep runs a full quarter ahead of the PE
        btq_pool = ctx.enter_context(tc.tile_pool(name="btq", bufs=3))
        ew_pool = ctx.enter_context(tc.tile_pool(name="ew", bufs=2))

        eps_tile = singles.tile([P, 1], F32)
        nc.vector.memset(eps_tile, EPS_NORM)
        ones_col = singles.tile([P, 1], F32)
        nc.vector.memset(ones_col, 1.0)

        # target_row broadcast to all partitions: [128, N] fp16 (exact ints)
        trow_bc = singles.tile([P, N], F16)
        trow_b = bass.AP(
            tensor=trow_ap.tensor,
            offset=trow_ap.offset,
            ap=[[0, P]] + list(trow_ap.ap),
        )
        nc.sync.dma_start(out=trow_bc, in_=trow_b)

        # per-chunk targets_col as per-partition scalars: [128, NCH] fp32
        tcol_sb = singles.tile([P, NCH], F32)
        tcol2 = tcol_ap.rearrange("(c p) -> c p", p=P)
        for c in range(NCH):
            nc.scalar.dma_start(out=tcol_sb[:, c : c + 1], in_=tcol2[c][:, None])

        # row-reduction strips, one column per (chunk, quarter)
        rqn_strip = singles.tile([P, NCH * NQ], F32)   # -pos
        nga_strip = singles.tile([P, NCH * NQ], F32)   # neg relu part
        ngb_strip = singles.tile([P, NCH * NQ], F32)   # neg m*count part

        # ---- A shard: normalize + blocked transpose -> AT [128, KT, ROWS]
        at_sb = singles.tile([P, KT, ROWS], BF16)
        for c in range(NCH):
            eng = dma_engines[c % 2]
            xf = stage_f32.tile([P, D], F32, tag="xf")
            eng.dma_start(out=xf, in_=a_ap[ds(c * P, P), :])
            xb = stage_bf.tile([P, D], BF16, tag="xb")
            _normalize_tile(nc, (small, junk), eng, xf, xb, eps_tile)
            dma_engines[(c + 1) % 2].dma_start_transpose(
                out=at_sb[:, :, ds(c * P, P)], in_=xb
            )

        # ---- main loop over B quarters
        psmm_ctx = tc.tile_pool(name="psmm", bufs=2, space=bass.MemorySpace.PSUM)
        psum_mm = psmm_ctx.__enter__()
        for q in range(NQ):
            bt = btq_pool.tile([P, KT, QJ], BF16, tag="bt")
            for t in range(QJ // P):  # 16 row-tiles per quarter
                row0 = q * QJ + t * P
                eng = dma_engines[t % 2]
                xf = stage_f32.tile([P, D], F32, tag="xf")
                eng.dma_start(out=xf, in_=b_ap[ds(row0, P), :])
                xb = stage_bf.tile([P, D], BF16, tag="xb")
                _normalize_tile(nc, (small, junk), eng, xf, xb, eps_tile)
                dma_engines[(t + 1) % 2].dma_start_transpose(
                    out=bt[:, :, ds(t * P, P)], in_=xb
                )

            for c in range(NCH):
                col = c * NQ + q
                # k-outer: 4 consecutive matmuls share one stationary
                # operand, so LDWEIGHTS amortizes 4x and PE stays fed
                ps = psum_mm.tile([P, QJ], F32, tag="ps")
                for k in range(KT):
                    for h in range(QJ // 512):
                        nc.tensor.matmul(
                            ps[:, ds(h * 512, 512)],
                            at_sb[:, k, ds(c * P, P)],
                            bt[:, k, ds(h * 512, 512)],
                            start=(k == 0),
                            stop=(k == KT - 1),
                        )
                # sole PSUM reader (frees banks): sim block -> bf16 SBUF
                smb = ew_pool.tile([P, QJ], BF16, tag="smb")
                nc.scalar.activation(smb, ps, AF.Copy, bias=0.0, scale=1.0)

                # nm = BIG * (trow != tcol_c)        [4x tensor_scalar]
                nm = ew_pool.tile([P, QJ], BF16, tag="nm")
                nc.vector.tensor_scalar(
                    out=nm,
                    in0=trow_bc[:, ds(q * QJ, QJ)],
                    scalar1=tcol_sb[:, c : c + 1],
                    scalar2=BIG,
                    op0=OP.not_equal,
                    op1=OP.mult,
                )
                # w = sim + nm                       [2x tensor_tensor]
                w = ew_pool.tile([P, QJ], BF16, tag="w")
                nc.vector.tensor_tensor(out=w, in0=smb, in1=nm, op=OP.add)
                # With accum_out, op1 is the REDUCTION op (add) — only op0
                # applies elementwise.  Clamp identities give single-op0
                # forms; the constant offsets are applied in the finalize:
                #   sum min(w-c1,0)       = sum min(w,c1)  - QJ*c1
                #   sum max(w-(m+BIG),0)  = sum max(w,m+B) - QJ*(m+BIG)
                # pos = sum relu(1 - w) on ACT (balances DVE/ACT load).
                # Exact: mismatch rows have 1-w <= -6.5 -> 0; the
                # reference's 1-1e-5 threshold is equivalent for any
                # sim outside [1-1e-5, 1).
                j1 = ew_pool.tile([P, QJ], BF16, tag="j", bufs=3)
                nc.scalar.activation(
                    j1, w, AF.Relu, bias=1.0, scale=-1.0,
                    accum_out=rqn_strip[:, col : col + 1],
                )
                j2 = ew_pool.tile([P, QJ], BF16, tag="j", bufs=3)
                nc.vector.tensor_scalar(
                    out=j2,
                    in0=w,
                    scalar1=MARGIN + BIG,
                    scalar2=None,
                    op0=OP.max,
                    op1=OP.add,
                    accum_out=nga_strip[:, col : col + 1],
                )
                # neg count: sum (w > m+BIG)  (times MARGIN in finalize)
                j3 = ew_pool.tile([P, QJ], BF16, tag="j", bufs=3)
                nc.vector.tensor_scalar(
                    out=j3,
                    in0=w,
                    scalar1=MARGIN + BIG,
                    scalar2=None,
                    op0=OP.is_gt,
                    op1=OP.add,
                    accum_out=ngb_strip[:, col : col + 1],
                )

        psmm_ctx.__exit__(None, None, None)

        # ---- finalize: row_loss = (pos > 0) * (pos + negA + negB) with
        #   pos  = sum_q rqn (relu(1-w) rowsums),
        #   negA = sum_q nga - N*(m+BIG),  negB = MARGIN * sum_q ngb
        loss_acc = singles.tile([P, 1], F32)
        nc.vector.memset(loss_acc, 0.0)
        for c in range(NCH):
            sl = ds(c * NQ, NQ)
            pos = small.tile([P, 1], F32, tag="pos")
            nc.vector.tensor_reduce(
                pos, rqn_strip[:, sl], axis=mybir.AxisListType.X, op=OP.add
            )
            nga = small.tile([P, 1], F32, tag="nga")
            nc.vector.tensor_reduce(
                nga, nga_strip[:, sl], axis=mybir.AxisListType.X, op=OP.add
            )
            ngb = small.tile([P, 1], F32, tag="ngb")
            nc.vector.tensor_reduce(
                ngb, ngb_strip[:, sl], axis=mybir.AxisListType.X, op=OP.add
            )
            neg = small.tile([P, 1], F32, tag="neg")
            nc.vector.tensor_scalar(
                out=neg, in0=nga, scalar1=N * (MARGIN + BIG), scalar2=None,
                op0=OP.subtract,
            )
            ngbm = small.tile([P, 1], F32, tag="ngbm")
            nc.vector.tensor_scalar(
                out=ngbm, in0=ngb, scalar1=MARGIN, scalar2=None, op0=OP.mult
            )
            ind = small.tile([P, 1], F32, tag="ind")
            nc.vector.tensor_scalar(
                out=ind, in0=pos, scalar1=0.0, scalar2=None, op0=OP.is_gt
            )
            tmp = small.tile([P, 1], F32, tag="tmp")
            nc.vector.tensor_add(tmp, neg, ngbm)
            nc.vector.tensor_add(tmp, tmp, pos)
            nc.vector.tensor_mul(tmp, tmp, ind)
            nc.vector.tensor_add(loss_acc, loss_acc, tmp)

        # partition-sum via a [128,1]x[128,1] matmul (PSUM is free again)
        with tc.tile_pool(
            name="psfin", bufs=1, space=bass.MemorySpace.PSUM
        ) as psum_fin:
            pfin = psum_fin.tile([1, 1], F32)
            nc.tensor.matmul(pfin, loss_acc, ones_col, start=True, stop=True)
            ob = small.tile([1, 1], F32, tag="ob")
            nc.vector.tensor_copy(ob, pfin)
            nc.sync.dma_start(out=out_ap, in_=ob)


_NC_CACHE = {}


def build_nc(reps=1):
    if reps in _NC_CACHE:
        return _NC_CACHE[reps]
    nc = bacc.Bacc("TRN2", target_bir_lowering=False, debug=False)
    a_ap = nc.dram_tensor("a_shard", [ROWS, D], F32, kind="ExternalInput").ap()
    b_ap = nc.dram_tensor("b_full", [N, D], F32, kind="ExternalInput").ap()
    tcol_ap = nc.dram_tensor("t_col", [ROWS], F32, kind="ExternalInput").ap()
    trow_ap = nc.dram_tensor("t_row", [N], F16, kind="ExternalInput").ap()
    out_ap = nc.dram_tensor("partial", [1, 1], F32, kind="ExternalOutput").ap()
    with tile.TileContext(nc) as tc:
        if reps == 1:
            build_kernel_body(tc, out_ap, a_ap, b_ap, tcol_ap, trow_ap)
        else:
            with tc.For_i(0, reps, 1):
                build_kernel_body(tc, out_ap, a_ap, b_ap, tcol_ap, trow_ap)
    nc.compile()
    _NC_CACHE[reps] = nc
    return nc


def make_in_maps(inputs_col, targets_col, inputs_row, target_row):
    b_full = np.ascontiguousarray(np.asarray(inputs_row, dtype=np.float32))
    trow = np.asarray(target_row).astype(np.float16)
    in_maps = []
    for c in range(NCORES):
        sl = slice(c * ROWS, (c + 1) * ROWS)
        in_maps.append(
            {
                "a_shard": np.ascontiguousarray(
                    np.asarray(inputs_col[sl], dtype=np.float32)
                ),
                "b_full": b_full,
                "t_col": np.asarray(targets_col[sl]).astype(np.float32),
                "t_row": trow,
            }
        )
    return in_maps


def kernel(**inputs):
    from concourse.bass_utils import run_bass_kernel_spmd

    nc = build_nc()
    in_maps = make_in_maps(
        inputs["inputs_col"],
        inputs["targets_col"],
        inputs["inputs_row"],
        inputs["target_row"],
    )
    res = run_bass_kernel_spmd(nc, in_maps, list(range(NCORES))).results
    total = 0.0
    for c in range(NCORES):
        total += float(res[c]["partial"][0, 0])
    return np.float32(total / N)
